# revision 3
# baseline (speedup 1.0000x reference)
"""Single-head encoder attention block on 8 Trainium2 NeuronCores.

Math (per batch element b):
    q = x @ wq.T ; k = x @ wk.T ; v = x @ wv.T
    scores = (q @ k.T) / sqrt(1024) ; attn = softmax(scores, -1)
    out = (attn @ v) @ wo.T

Sharding: data-parallel over batch — batch 8 maps 1:1 onto the 8 cores;
no collectives.

Weight folding (host, one-time input transformation):
    m  = (wq.T @ wk) / 32          scores   = x m x.T
    ut = (wo @ wv).T               out      = attn @ x @ ut
x, m, ut are cast to bf16 on host; xT is produced by the DMA crossbar
transpose (InstDmaTransposeAnt) so the PE does zero transposes.

Per-core device algorithm (bf16 operands, fp32 PSUM):
  A: xT via DMA-transpose (resident [d,s]), Z = x@ut (resident [j,do]),
     F = (x m).T (resident [d2,i]); no DRAM spills.
  B: per i-superblock of 512:
     scoresT[j,i] = xT.T F ; e = exp(scoresT)          (ACT)
     Tacc = sum_jtiles e                               (DVE)
     R = partition_all_reduce(Tacc)                    (GPSIMD)
     out[i,do] = sum_j e[j,i-tile] Z[j,do]             (PE)
     rc = 1/R via tiny PE transposes + DVE recip; out evict scaled by rc.
"""

import os
import sys

for _p in ("/opt/trn_rl_repo", "/root/.axon_site/_ro/trn_rl_repo"):
    if os.path.isdir(_p) and _p not in sys.path:
        sys.path.insert(0, _p)

import numpy as np
from contextlib import ExitStack

import concourse.bacc as bacc
import concourse.tile as tile
from concourse import mybir
from concourse.bass import bass_isa
from concourse.bass_utils import run_bass_kernel_spmd

P = 128
S = 2048          # sequence length (per core)
D = 1024          # model dim = dk = dv
NS = S // P       # 16 seq tiles
ND = D // P       # 8 dim tiles
SB = 512          # i-superblock width (query columns per block)
NSB = S // SB     # 4 superblocks
NIT = SB // P     # 4 i-tiles per superblock
N_CORES = 8

F32 = mybir.dt.float32
BF = mybir.dt.bfloat16
EXP = mybir.ActivationFunctionType.Exp
COPY = mybir.ActivationFunctionType.Copy


def _build():
    nc = bacc.Bacc("TRN2", target_bir_lowering=False, debug=False, num_devices=N_CORES)

    x_in = nc.dram_tensor("x", [S, D], BF, kind="ExternalInput").ap()
    m_in = nc.dram_tensor("m", [D, D], BF, kind="ExternalInput").ap()
    u_in = nc.dram_tensor("ut", [D, D], BF, kind="ExternalInput").ap()
    out_d = nc.dram_tensor("out", [S, D], F32, kind="ExternalOutput").ap()

    mm = nc.tensor.matmul

    with tile.TileContext(nc) as tc, ExitStack() as top:
        cst = top.enter_context(tc.tile_pool(name="cst", bufs=1))
        ones_f32 = cst.tile([P, 1], F32)
        nc.gpsimd.memset(ones_f32[:], 1.0)

        res1 = top.enter_context(tc.tile_pool(name="res1", bufs=1))
        xt = res1.tile([P, ND * S], BF)    # xT: d-tile t -> [:, t*S:(t+1)*S] = [d-part, s]
        res2 = top.enter_context(tc.tile_pool(name="res2", bufs=1))
        zres = res2.tile([P, NS * D], BF)  # Z: j-tile -> [:, j*D:(j+1)*D] = [j-part, do]
        res3 = top.enter_context(tc.tile_pool(name="res3", bufs=1))
        fres = res3.tile([P, ND * S], BF)  # F: d2-tile -> [:, t*S:(t+1)*S] = [d2-part, i]
        wres = top.enter_context(tc.tile_pool(name="wres", bufs=1))
        un = wres.tile([P, ND * D], BF)    # ut natural: d-tile -> [:, t*D:(t+1)*D] = [d-part, do]
        mn = wres.tile([P, ND * D], BF)    # m natural:  d1-tile -> [:, t*D:(t+1)*D] = [d1-part, d2]

        # ---------------- DMA issue (order = queue priority) ----------------
        # ALL xT transposes go on the SP queue: concurrent InstDmaTransposeAnt
        # on different queues corrupt each other (shared xbar); one queue's
        # ring serializes them. Normal DMAs on the ACT queue are safe.
        # s-chunk-major order so the Z j-loop can start after 8 transposes.
        for sc in range(4):
            for d in range(ND):
                nc.sync.dma_start_transpose(
                    out=xt[:, d * S + sc * 512: d * S + (sc + 1) * 512],
                    in_=x_in[sc * 512:(sc + 1) * 512, d * P:(d + 1) * P])
        for h in range(2):
            for d in range(ND):
                nc.scalar.dma_start(
                    out=un[:, d * D + h * 512: d * D + (h + 1) * 512],
                    in_=u_in[d * P:(d + 1) * P, h * 512:(h + 1) * 512])
        for t in range(ND):
            nc.scalar.dma_start(out=mn[:, t * D:(t + 1) * D],
                                in_=m_in[t * P:(t + 1) * P, :])

        # ---------------- Phase A: Z then F (PSUM pool closed after) --------
        with ExitStack() as pa:
            mmps = pa.enter_context(tc.tile_pool(name="mmps", bufs=8, space="PSUM"))

            # Z[j, do] = sum_d xT[d, j].T ut[d, do]
            for j in range(NS):
                zp0 = mmps.tile([P, 512], F32, name="zp0", tag="mm")
                zp1 = mmps.tile([P, 512], F32, name="zp1", tag="mm")
                for d in range(ND):
                    stat = xt[:, d * S + j * P: d * S + (j + 1) * P]
                    mm(zp0[:], stat, un[:, d * D: d * D + 512],
                       start=(d == 0), stop=(d == ND - 1))
                    mm(zp1[:], stat, un[:, d * D + 512: (d + 1) * D],
                       start=(d == 0), stop=(d == ND - 1))
                nc.scalar.copy(zres[:, j * D: j * D + 512], zp0[:])
                nc.scalar.copy(zres[:, j * D + 512: (j + 1) * D], zp1[:])

            # F[d2, i] = sum_d1 m[d1, d2].T xT[d1, i]
            for t2 in range(ND):
                fps = [mmps.tile([P, 512], F32, name=f"fp{ic}", tag="mm")
                       for ic in range(4)]
                for t1 in range(ND):
                    stat = mn[:, t1 * D + t2 * P: t1 * D + (t2 + 1) * P]
                    for ic in range(4):
                        mm(fps[ic][:], stat, xt[:, t1 * S + ic * 512: t1 * S + (ic + 1) * 512],
                           start=(t1 == 0), stop=(t1 == ND - 1))
                for ic in range(4):
                    nc.scalar.copy(fres[:, t2 * S + ic * 512: t2 * S + (ic + 1) * 512],
                                   fps[ic][:])

        # ---------------- Phase B ----------------
        with ExitStack() as pb:
            scps = pb.enter_context(tc.tile_pool(name="scps", bufs=3, space="PSUM"))
            outps = pb.enter_context(tc.tile_pool(name="outps", bufs=3, space="PSUM"))
            miscps = pb.enter_context(tc.tile_pool(name="miscps", bufs=2, space="PSUM"))
            expp = pb.enter_context(tc.tile_pool(name="expp", bufs=18))
            taccp = pb.enter_context(tc.tile_pool(name="taccp", bufs=2))
            rbp = pb.enter_context(tc.tile_pool(name="rbp", bufs=2))
            rcp = pb.enter_context(tc.tile_pool(name="rcp", bufs=8))
            outsb = pb.enter_context(tc.tile_pool(name="outsb", bufs=3))

            for sbi in range(NSB):
                # scoresT[j, i] + exp, with DVE rowsum accumulation chasing
                tacc = taccp.tile([P, SB], F32, name="tacc", tag="ta")
                ets = []
                for j in range(NS):
                    sc = scps.tile([P, SB], F32, tag="sc")
                    for t2 in range(ND):
                        mm(sc[:],
                           xt[:, t2 * S + j * P: t2 * S + (j + 1) * P],
                           fres[:, t2 * S + sbi * SB: t2 * S + (sbi + 1) * SB],
                           start=(t2 == 0), stop=(t2 == ND - 1))
                    et = expp.tile([P, SB], BF, name=f"et{j}", tag="et")
                    nc.scalar.activation(et[:], sc[:], EXP)
                    ets.append(et)
                    if j == 0:
                        nc.vector.tensor_copy(tacc[:], et[:])
                    else:
                        nc.vector.tensor_add(tacc[:], tacc[:], et[:])

                # R[i] broadcast across partitions (GPSIMD, off the PE path)
                rbc = rbp.tile([P, SB], F32, name="rbc", tag="rb")
                nc.gpsimd.partition_all_reduce(rbc[:], tacc[:], P,
                                               bass_isa.ReduceOp.add)

                # out[i, do] = sum_j e[j, i-tile].T Z[j, do]
                recips = [None] * NIT
                for it in range(NIT):
                    op0 = outps.tile([P, 512], F32, name="op0", tag="op")
                    op1 = outps.tile([P, 512], F32, name="op1", tag="op")
                    for j in range(NS):
                        stat = ets[j][:, it * P:(it + 1) * P]
                        mm(op0[:], stat, zres[:, j * D: j * D + 512],
                           start=(j == 0), stop=(j == NS - 1))
                        mm(op1[:], stat, zres[:, j * D + 512: (j + 1) * D],
                           start=(j == 0), stop=(j == NS - 1))
                    if it == 0:
                        # per-partition 1/R tiles; PE reaches this ~7us after
                        # the last exp, so rbc is long ready
                        for it2 in range(NIT):
                            tp = miscps.tile([P, 1], F32, name=f"rtp{it2}", tag="m")
                            nc.tensor.transpose(tp[:], rbc[0:1, it2 * P:(it2 + 1) * P],
                                                ones_f32[0:1, 0:1])
                            rc = rcp.tile([P, 1], F32, name=f"rc{it2}", tag="rc")
                            nc.vector.reciprocal(rc[:], tp[:])
                            recips[it2] = rc
                    row = (sbi * NIT + it) * P
                    for ch, op in ((0, op0), (1, op1)):
                        ob = outsb.tile([P, 512], F32, tag="ob")
                        nc.scalar.activation(ob[:], op[:], COPY,
                                             scale=recips[it][:, 0:1])
                        nc.sync.dma_start(
                            out=out_d[row:row + P, ch * 512:(ch + 1) * 512],
                            in_=ob[:])

    nc.compile()
    return nc


_NC_CACHE = None


def _bf16(a):
    import ml_dtypes
    return np.asarray(a, dtype=np.float32).astype(ml_dtypes.bfloat16)


def kernel(x, wq, wk, wv, wo):
    global _NC_CACHE
    if _NC_CACHE is None:
        _NC_CACHE = _build()
    nc = _NC_CACHE

    # Host-side weight folding (one-time input transformation) + bf16 casts.
    wq32 = np.asarray(wq, dtype=np.float32)
    wk32 = np.asarray(wk, dtype=np.float32)
    wv32 = np.asarray(wv, dtype=np.float32)
    wo32 = np.asarray(wo, dtype=np.float32)
    m_bf = _bf16((wq32.T @ wk32) / 32.0)
    ut_bf = _bf16((wo32 @ wv32).T)

    core_ids = list(range(N_CORES))
    in_maps = []
    for b in range(N_CORES):
        in_maps.append({
            "x": _bf16(x[b]),
            "m": m_bf,
            "ut": ut_bf,
        })
    res = run_bass_kernel_spmd(nc, in_maps, core_ids)
    return np.stack([res.results[b]["out"] for b in range(N_CORES)], axis=0)


# revision 5
# speedup vs baseline: 1.4509x; 1.4509x over previous
"""Single-head encoder attention block on 8 Trainium2 NeuronCores.

Math (per batch element b):
    q = x @ wq.T ; k = x @ wk.T ; v = x @ wv.T
    scores = (q @ k.T) / sqrt(1024) ; attn = softmax(scores, -1)
    out = (attn @ v) @ wo.T

Sharding: data-parallel over batch — batch 8 maps 1:1 onto the 8 cores;
no collectives.

Weight folding (host, one-time input transformation):
    m  = (wq.T @ wk) / 32          scores   = x m x.T
    ut = (wo @ wv).T               out      = attn @ x @ ut
m, ut are cast to bf16 on host and loaded in natural layout (no device
transposes needed for weights).

Per-core device algorithm (bf16 matmul operands, fp32 PSUM):
  A: xT via PE is_transpose matmuls (fp32r, 1.5 c/row), evicted to bf16,
     interleaved with Z = x@ut so the PE never idles waiting on DMA;
     then F = (x m).T. xT, Z, F all SBUF-resident (no DRAM spills).
  B: per i-superblock of 512:
     scoresT[j,i] = xT.T F ; e = exp(scoresT)          (ACT)
     Tacc = sum_jtiles e                               (DVE)
     R = partition_all_reduce(Tacc)                    (GPSIMD)
     out[i,do] = sum_j e[j,i-tile] Z[j,do]             (PE)
     rc = 1/R via tiny PE transposes + DVE recip; out evict scaled by rc.
"""

import os
import sys

for _p in ("/opt/trn_rl_repo", "/root/.axon_site/_ro/trn_rl_repo"):
    if os.path.isdir(_p) and _p not in sys.path:
        sys.path.insert(0, _p)

import numpy as np
from contextlib import ExitStack

import concourse.bacc as bacc
import concourse.tile as tile
from concourse import mybir, masks
from concourse.bass import bass_isa
from concourse.bass_utils import run_bass_kernel_spmd

P = 128
S = 2048          # sequence length (per core)
D = 1024          # model dim = dk = dv
NS = S // P       # 16 seq tiles
ND = D // P       # 8 dim tiles
SB = 512          # i-superblock width (query columns per block)
NSB = S // SB     # 4 superblocks
NIT = SB // P     # 4 i-tiles per superblock
N_CORES = 8

F32 = mybir.dt.float32
F32R = mybir.dt.float32r
BF = mybir.dt.bfloat16
EXP = mybir.ActivationFunctionType.Exp
COPY = mybir.ActivationFunctionType.Copy


def _build():
    nc = bacc.Bacc("TRN2", target_bir_lowering=False, debug=False, num_devices=N_CORES)

    x_in = nc.dram_tensor("x", [S, D], F32, kind="ExternalInput").ap()
    m_in = nc.dram_tensor("m", [D, D], BF, kind="ExternalInput").ap()
    u_in = nc.dram_tensor("ut", [D, D], BF, kind="ExternalInput").ap()
    out_d = nc.dram_tensor("out", [S, D], F32, kind="ExternalOutput").ap()

    mm = nc.tensor.matmul

    with tile.TileContext(nc) as tc, ExitStack() as top:
        cst = top.enter_context(tc.tile_pool(name="cst", bufs=1))
        ident_f32 = cst.tile([P, P], F32)
        masks.make_identity(nc, ident_f32[:])
        ident = cst.tile([P, P], F32R)
        nc.vector.tensor_copy(ident[:], ident_f32[:])

        res1 = top.enter_context(tc.tile_pool(name="res1", bufs=1))
        xt = res1.tile([P, ND * S], BF)    # xT: d-tile t -> [:, t*S:(t+1)*S] = [d-part, s]
        res2 = top.enter_context(tc.tile_pool(name="res2", bufs=1))
        zres = res2.tile([P, NS * D], BF)  # Z: j-tile -> [:, j*D:(j+1)*D] = [j-part, do]
        res3 = top.enter_context(tc.tile_pool(name="res3", bufs=1))
        fres = res3.tile([P, ND * S], BF)  # F: d2-tile -> [:, t*S:(t+1)*S] = [d2-part, i]
        wres = top.enter_context(tc.tile_pool(name="wres", bufs=1))
        un = wres.tile([P, ND * D], BF)    # ut natural: d-tile -> [:, t*D:(t+1)*D] = [d-part, do]
        mn = wres.tile([P, ND * D], BF)    # m natural:  d1-tile -> [:, t*D:(t+1)*D] = [d1-part, d2]

        # ---------------- DMA issue (order = queue priority) ----------------
        # x row-tiles (f32, halves for latency) on the SP queue; un/mn on ACT.
        xstg = top.enter_context(tc.tile_pool(name="xstg", bufs=6))
        x_tiles = []
        for j in range(NS):
            xs = xstg.tile([P, D], F32R, name=f"xs{j}", tag="xs")
            for hf in range(2):
                nc.sync.dma_start(
                    out=xs[:, hf * 512:(hf + 1) * 512],
                    in_=x_in[j * P:(j + 1) * P, hf * 512:(hf + 1) * 512].bitcast(F32R))
            x_tiles.append(xs)
        for d in range(ND):
            for h in range(2):
                nc.scalar.dma_start(
                    out=un[:, d * D + h * 512: d * D + (h + 1) * 512],
                    in_=u_in[d * P:(d + 1) * P, h * 512:(h + 1) * 512])
        for t in range(ND):
            nc.scalar.dma_start(out=mn[:, t * D:(t + 1) * D],
                                in_=m_in[t * P:(t + 1) * P, :])

        # ---------------- Phase A: xT (PE transposes) + Z, then F -----------
        with ExitStack() as pa:
            mmps = pa.enter_context(tc.tile_pool(name="mmps", bufs=8, space="PSUM"))

            def transpose_j(j):
                """xT blocks for s-tile j: two [128,512] f32r psum tiles
                (d0-3, d4-7), evicted as 8 [128,128] bf16 copies on DVE."""
                xs = x_tiles[j]
                for g in range(2):
                    tp = mmps.tile([P, 512], F32R, name=f"tp{g}", tag="mm")
                    for k in range(4):
                        d = g * 4 + k
                        nc.tensor.transpose(tp[:, k * P:(k + 1) * P],
                                            xs[:, d * P:(d + 1) * P], ident[:])
                    for k in range(4):
                        d = g * 4 + k
                        nc.vector.tensor_copy(
                            xt[:, d * S + j * P: d * S + (j + 1) * P],
                            tp[:, k * P:(k + 1) * P])

            def z_j(j):
                zp0 = mmps.tile([P, 512], F32, name="zp0", tag="mm")
                zp1 = mmps.tile([P, 512], F32, name="zp1", tag="mm")
                for d in range(ND):
                    stat = xt[:, d * S + j * P: d * S + (j + 1) * P]
                    mm(zp0[:], stat, un[:, d * D: d * D + 512],
                       start=(d == 0), stop=(d == ND - 1))
                    mm(zp1[:], stat, un[:, d * D + 512: (d + 1) * D],
                       start=(d == 0), stop=(d == ND - 1))
                nc.scalar.copy(zres[:, j * D: j * D + 512], zp0[:])
                nc.scalar.copy(zres[:, j * D + 512: (j + 1) * D], zp1[:])

            # interleave: transposes chase the x DMAs, Z trails by one j
            for j in range(NS):
                transpose_j(j)
                if j >= 1:
                    z_j(j - 1)
            z_j(NS - 1)

            # F[d2, i] = sum_d1 m[d1, d2].T xT[d1, i]
            for t2 in range(ND):
                fps = [mmps.tile([P, 512], F32, name=f"fp{ic}", tag="mm")
                       for ic in range(4)]
                for t1 in range(ND):
                    stat = mn[:, t1 * D + t2 * P: t1 * D + (t2 + 1) * P]
                    for ic in range(4):
                        mm(fps[ic][:], stat, xt[:, t1 * S + ic * 512: t1 * S + (ic + 1) * 512],
                           start=(t1 == 0), stop=(t1 == ND - 1))
                for ic in range(4):
                    nc.scalar.copy(fres[:, t2 * S + ic * 512: t2 * S + (ic + 1) * 512],
                                   fps[ic][:])

        # ---------------- Phase B ----------------
        with ExitStack() as pb:
            scps = pb.enter_context(tc.tile_pool(name="scps", bufs=3, space="PSUM"))
            outps = pb.enter_context(tc.tile_pool(name="outps", bufs=3, space="PSUM"))
            miscps = pb.enter_context(tc.tile_pool(name="miscps", bufs=2, space="PSUM"))
            expp = pb.enter_context(tc.tile_pool(name="expp", bufs=18))
            taccp = pb.enter_context(tc.tile_pool(name="taccp", bufs=2))
            rbp = pb.enter_context(tc.tile_pool(name="rbp", bufs=2))
            rcp = pb.enter_context(tc.tile_pool(name="rcp", bufs=8))
            outsb = pb.enter_context(tc.tile_pool(name="outsb", bufs=3))

            for sbi in range(NSB):
                # scoresT[j, i] + exp, with DVE rowsum accumulation chasing
                tacc = taccp.tile([P, SB], F32, name="tacc", tag="ta")
                ets = []
                for j in range(NS):
                    sc = scps.tile([P, SB], F32, tag="sc")
                    for t2 in range(ND):
                        mm(sc[:],
                           xt[:, t2 * S + j * P: t2 * S + (j + 1) * P],
                           fres[:, t2 * S + sbi * SB: t2 * S + (sbi + 1) * SB],
                           start=(t2 == 0), stop=(t2 == ND - 1))
                    et = expp.tile([P, SB], BF, name=f"et{j}", tag="et")
                    nc.scalar.activation(et[:], sc[:], EXP)
                    ets.append(et)
                    if j == 0:
                        nc.vector.tensor_copy(tacc[:], et[:])
                    else:
                        nc.vector.tensor_add(tacc[:], tacc[:], et[:])

                # R[i] broadcast across partitions (GPSIMD, off the PE path)
                rbc = rbp.tile([P, SB], F32, name="rbc", tag="rb")
                nc.gpsimd.partition_all_reduce(rbc[:], tacc[:], P,
                                               bass_isa.ReduceOp.add)

                # out[i, do] = sum_j e[j, i-tile].T Z[j, do]
                recips = [None] * NIT
                for it in range(NIT):
                    op0 = outps.tile([P, 512], F32, name="op0", tag="op")
                    op1 = outps.tile([P, 512], F32, name="op1", tag="op")
                    for j in range(NS):
                        stat = ets[j][:, it * P:(it + 1) * P]
                        mm(op0[:], stat, zres[:, j * D: j * D + 512],
                           start=(j == 0), stop=(j == NS - 1))
                        mm(op1[:], stat, zres[:, j * D + 512: (j + 1) * D],
                           start=(j == 0), stop=(j == NS - 1))
                    if it == 0:
                        # per-partition 1/R tiles; PE reaches this ~7us after
                        # the last exp, so rbc is long ready
                        for it2 in range(NIT):
                            tp = miscps.tile([P, 1], F32, name=f"rtp{it2}", tag="m")
                            nc.tensor.transpose(tp[:], rbc[0:1, it2 * P:(it2 + 1) * P],
                                                ident_f32[0:1, 0:1])
                            rc = rcp.tile([P, 1], F32, name=f"rc{it2}", tag="rc")
                            nc.vector.reciprocal(rc[:], tp[:])
                            recips[it2] = rc
                    row = (sbi * NIT + it) * P
                    for ch, op in ((0, op0), (1, op1)):
                        ob = outsb.tile([P, 512], F32, tag="ob")
                        nc.scalar.activation(ob[:], op[:], COPY,
                                             scale=recips[it][:, 0:1])
                        nc.sync.dma_start(
                            out=out_d[row:row + P, ch * 512:(ch + 1) * 512],
                            in_=ob[:])

    nc.compile()
    return nc


_NC_CACHE = None


def _bf16(a):
    import ml_dtypes
    return np.asarray(a, dtype=np.float32).astype(ml_dtypes.bfloat16)


def kernel(x, wq, wk, wv, wo):
    global _NC_CACHE
    if _NC_CACHE is None:
        _NC_CACHE = _build()
    nc = _NC_CACHE

    # Host-side weight folding (one-time input transformation) + bf16 casts.
    wq32 = np.asarray(wq, dtype=np.float32)
    wk32 = np.asarray(wk, dtype=np.float32)
    wv32 = np.asarray(wv, dtype=np.float32)
    wo32 = np.asarray(wo, dtype=np.float32)
    m_bf = _bf16((wq32.T @ wk32) / 32.0)
    ut_bf = _bf16((wo32 @ wv32).T)

    core_ids = list(range(N_CORES))
    in_maps = []
    for b in range(N_CORES):
        in_maps.append({
            "x": np.ascontiguousarray(x[b], dtype=np.float32),
            "m": m_bf,
            "ut": ut_bf,
        })
    res = run_bass_kernel_spmd(nc, in_maps, core_ids)
    return np.stack([res.results[b]["out"] for b in range(N_CORES)], axis=0)


# revision 7
# speedup vs baseline: 1.4636x; 1.0088x over previous
"""Single-head encoder attention block on 8 Trainium2 NeuronCores.

Math (per batch element b):
    q = x @ wq.T ; k = x @ wk.T ; v = x @ wv.T
    scores = (q @ k.T) / sqrt(1024) ; attn = softmax(scores, -1)
    out = (attn @ v) @ wo.T

Sharding: data-parallel over batch — batch 8 maps 1:1 onto the 8 cores;
no collectives.

Weight folding (host, one-time input transformation):
    m  = (wq.T @ wk) / 32          scores   = x m x.T
    ut = (wo @ wv).T               out      = attn @ x @ ut
m, ut are cast to bf16 on host and loaded in natural layout (no device
transposes needed for weights).

Per-core device algorithm (bf16 matmul operands, fp32 PSUM):
  A: xT via PE is_transpose matmuls (fp32r, 1.5 c/row), evicted to bf16,
     interleaved with Z = x@ut so the PE never idles waiting on DMA;
     then F = (x m).T. xT, Z, F all SBUF-resident (no DRAM spills).
  B: per i-superblock of 512:
     scoresT[j,i] = xT.T F ; e = exp(scoresT)          (ACT)
     Tacc = sum_jtiles e                               (DVE)
     R = partition_all_reduce(Tacc)                    (GPSIMD)
     out[i,do] = sum_j e[j,i-tile] Z[j,do]             (PE)
     rc = 1/R via tiny PE transposes + DVE recip; out evict scaled by rc.
"""

import os
import sys

for _p in ("/opt/trn_rl_repo", "/root/.axon_site/_ro/trn_rl_repo"):
    if os.path.isdir(_p) and _p not in sys.path:
        sys.path.insert(0, _p)

import numpy as np
from contextlib import ExitStack

import concourse.bacc as bacc
import concourse.tile as tile
from concourse import mybir, masks
from concourse.bass import bass_isa
from concourse.bass_utils import run_bass_kernel_spmd

P = 128
S = 2048          # sequence length (per core)
D = 1024          # model dim = dk = dv
NS = S // P       # 16 seq tiles
ND = D // P       # 8 dim tiles
SB = 512          # i-superblock width (query columns per block)
NSB = S // SB     # 4 superblocks
NIT = SB // P     # 4 i-tiles per superblock
N_CORES = 8

F32 = mybir.dt.float32
F32R = mybir.dt.float32r
BF = mybir.dt.bfloat16
EXP = mybir.ActivationFunctionType.Exp
COPY = mybir.ActivationFunctionType.Copy


def _build():
    nc = bacc.Bacc("TRN2", target_bir_lowering=False, debug=False, num_devices=N_CORES)

    x_in = nc.dram_tensor("x", [S, D], F32, kind="ExternalInput").ap()
    m_in = nc.dram_tensor("m", [D, D], BF, kind="ExternalInput").ap()
    u_in = nc.dram_tensor("ut", [D, D], BF, kind="ExternalInput").ap()
    out_d = nc.dram_tensor("out", [S, D], F32, kind="ExternalOutput").ap()

    mm = nc.tensor.matmul

    with tile.TileContext(nc) as tc, ExitStack() as top:
        cst = top.enter_context(tc.tile_pool(name="cst", bufs=1))
        ident_f32 = cst.tile([P, P], F32)
        masks.make_identity(nc, ident_f32[:])
        ident = cst.tile([P, P], F32R)
        nc.vector.tensor_copy(ident[:], ident_f32[:])

        res1 = top.enter_context(tc.tile_pool(name="res1", bufs=1))
        xt = res1.tile([P, ND * S], BF)    # xT: d-tile t -> [:, t*S:(t+1)*S] = [d-part, s]
        res2 = top.enter_context(tc.tile_pool(name="res2", bufs=1))
        zres = res2.tile([P, NS * D], BF)  # Z: j-tile -> [:, j*D:(j+1)*D] = [j-part, do]
        res3 = top.enter_context(tc.tile_pool(name="res3", bufs=1))
        fres = res3.tile([P, ND * S], BF)  # F: d2-tile -> [:, t*S:(t+1)*S] = [d2-part, i]
        wres = top.enter_context(tc.tile_pool(name="wres", bufs=1))
        un = wres.tile([P, ND * D], BF)    # ut natural: d-tile -> [:, t*D:(t+1)*D] = [d-part, do]
        mn = wres.tile([P, ND * D], BF)    # m natural:  d1-tile -> [:, t*D:(t+1)*D] = [d1-part, d2]

        # ---------------- DMA issue (order = queue priority) ----------------
        # x row-tiles (f32) on the SP queue, mn behind them; un on the ACT
        # queue (few issues so ACT is free for evicts by ~6us).
        xstg = top.enter_context(tc.tile_pool(name="xstg", bufs=6))
        x_tiles = []
        for j in range(NS):
            xs = xstg.tile([P, D], F32R, name=f"xs{j}", tag="xs")
            if j == 0:
                for hf in range(2):
                    nc.sync.dma_start(
                        out=xs[:, hf * 512:(hf + 1) * 512],
                        in_=x_in[:P, hf * 512:(hf + 1) * 512].bitcast(F32R))
            else:
                nc.sync.dma_start(out=xs[:],
                                  in_=x_in[j * P:(j + 1) * P, :].bitcast(F32R))
            x_tiles.append(xs)
        for d in range(ND):
            nc.scalar.dma_start(out=un[:, d * D:(d + 1) * D],
                                in_=u_in[d * P:(d + 1) * P, :])
        for t in range(ND):
            nc.sync.dma_start(out=mn[:, t * D:(t + 1) * D],
                              in_=m_in[t * P:(t + 1) * P, :])

        # ---------------- Phase A: xT (PE transposes) + Z, then F -----------
        with ExitStack() as pa:
            mmps = pa.enter_context(tc.tile_pool(name="mmps", bufs=8, space="PSUM"))

            def transpose_j(j):
                """xT blocks for s-tile j: two [128,512] f32r psum tiles
                (d0-3, d4-7), evicted as 8 [128,128] bf16 copies on DVE."""
                xs = x_tiles[j]
                for g in range(2):
                    tp = mmps.tile([P, 512], F32R, name=f"tp{g}", tag="mm")
                    for k in range(4):
                        d = g * 4 + k
                        nc.tensor.transpose(tp[:, k * P:(k + 1) * P],
                                            xs[:, d * P:(d + 1) * P], ident[:])
                    for k in range(4):
                        d = g * 4 + k
                        nc.vector.tensor_copy(
                            xt[:, d * S + j * P: d * S + (j + 1) * P],
                            tp[:, k * P:(k + 1) * P])

            def z_j(j):
                zp0 = mmps.tile([P, 512], F32, name="zp0", tag="mm")
                zp1 = mmps.tile([P, 512], F32, name="zp1", tag="mm")
                for d in range(ND):
                    stat = xt[:, d * S + j * P: d * S + (j + 1) * P]
                    mm(zp0[:], stat, un[:, d * D: d * D + 512],
                       start=(d == 0), stop=(d == ND - 1))
                    mm(zp1[:], stat, un[:, d * D + 512: (d + 1) * D],
                       start=(d == 0), stop=(d == ND - 1))
                nc.scalar.copy(zres[:, j * D: j * D + 512], zp0[:])
                nc.scalar.copy(zres[:, j * D + 512: (j + 1) * D], zp1[:])

            # interleave: transposes chase the x DMAs, Z trails by two j
            for j in range(NS):
                transpose_j(j)
                if j >= 2:
                    z_j(j - 2)
            z_j(NS - 2)
            z_j(NS - 1)

            # F[d2, i] = sum_d1 m[d1, d2].T xT[d1, i]
            # ic-outer so phase B's superblock sbi=ic never waits on late F
            for ic in range(4):
                for t2 in range(ND):
                    fp = mmps.tile([P, 512], F32, name="fp", tag="mm")
                    for t1 in range(ND):
                        stat = mn[:, t1 * D + t2 * P: t1 * D + (t2 + 1) * P]
                        mm(fp[:], stat, xt[:, t1 * S + ic * 512: t1 * S + (ic + 1) * 512],
                           start=(t1 == 0), stop=(t1 == ND - 1))
                    nc.scalar.copy(fres[:, t2 * S + ic * 512: t2 * S + (ic + 1) * 512],
                                   fp[:])

        # ---------------- Phase B ----------------
        with ExitStack() as pb:
            scps = pb.enter_context(tc.tile_pool(name="scps", bufs=3, space="PSUM"))
            outps = pb.enter_context(tc.tile_pool(name="outps", bufs=3, space="PSUM"))
            miscps = pb.enter_context(tc.tile_pool(name="miscps", bufs=2, space="PSUM"))
            expp = pb.enter_context(tc.tile_pool(name="expp", bufs=18))
            taccp = pb.enter_context(tc.tile_pool(name="taccp", bufs=2))
            rbp = pb.enter_context(tc.tile_pool(name="rbp", bufs=2))
            rcp = pb.enter_context(tc.tile_pool(name="rcp", bufs=8))
            outsb = pb.enter_context(tc.tile_pool(name="outsb", bufs=3))

            for sbi in range(NSB):
                # scoresT[j, i] + exp, with DVE rowsum accumulation chasing
                tacc = taccp.tile([P, SB], F32, name="tacc", tag="ta")
                ets = []
                for j in range(NS):
                    sc = scps.tile([P, SB], F32, tag="sc")
                    for t2 in range(ND):
                        mm(sc[:],
                           xt[:, t2 * S + j * P: t2 * S + (j + 1) * P],
                           fres[:, t2 * S + sbi * SB: t2 * S + (sbi + 1) * SB],
                           start=(t2 == 0), stop=(t2 == ND - 1))
                    et = expp.tile([P, SB], BF, name=f"et{j}", tag="et")
                    nc.scalar.activation(et[:], sc[:], EXP)
                    ets.append(et)
                    if j == 0:
                        nc.vector.tensor_copy(tacc[:], et[:])
                    else:
                        nc.vector.tensor_add(tacc[:], tacc[:], et[:])

                # R[i] broadcast across partitions (GPSIMD, off the PE path)
                rbc = rbp.tile([P, SB], F32, name="rbc", tag="rb")
                nc.gpsimd.partition_all_reduce(rbc[:], tacc[:], P,
                                               bass_isa.ReduceOp.add)

                # out[i, do] = sum_j e[j, i-tile].T Z[j, do]
                recips = [None] * NIT
                for it in range(NIT):
                    op0 = outps.tile([P, 512], F32, name="op0", tag="op")
                    op1 = outps.tile([P, 512], F32, name="op1", tag="op")
                    for j in range(NS):
                        stat = ets[j][:, it * P:(it + 1) * P]
                        mm(op0[:], stat, zres[:, j * D: j * D + 512],
                           start=(j == 0), stop=(j == NS - 1))
                        mm(op1[:], stat, zres[:, j * D + 512: (j + 1) * D],
                           start=(j == 0), stop=(j == NS - 1))
                    if it == 0:
                        # per-partition 1/R tiles; PE reaches this ~7us after
                        # the last exp, so rbc is long ready
                        for it2 in range(NIT):
                            tp = miscps.tile([P, 1], F32, name=f"rtp{it2}", tag="m")
                            nc.tensor.transpose(tp[:], rbc[0:1, it2 * P:(it2 + 1) * P],
                                                ident_f32[0:1, 0:1])
                            rc = rcp.tile([P, 1], F32, name=f"rc{it2}", tag="rc")
                            nc.vector.reciprocal(rc[:], tp[:])
                            recips[it2] = rc
                    row = (sbi * NIT + it) * P
                    for ch, op in ((0, op0), (1, op1)):
                        ob = outsb.tile([P, 512], F32, tag="ob")
                        nc.scalar.activation(ob[:], op[:], COPY,
                                             scale=recips[it][:, 0:1])
                        nc.sync.dma_start(
                            out=out_d[row:row + P, ch * 512:(ch + 1) * 512],
                            in_=ob[:])

    nc.compile()
    return nc


_NC_CACHE = None


def _bf16(a):
    import ml_dtypes
    return np.asarray(a, dtype=np.float32).astype(ml_dtypes.bfloat16)


def kernel(x, wq, wk, wv, wo):
    global _NC_CACHE
    if _NC_CACHE is None:
        _NC_CACHE = _build()
    nc = _NC_CACHE

    # Host-side weight folding (one-time input transformation) + bf16 casts.
    wq32 = np.asarray(wq, dtype=np.float32)
    wk32 = np.asarray(wk, dtype=np.float32)
    wv32 = np.asarray(wv, dtype=np.float32)
    wo32 = np.asarray(wo, dtype=np.float32)
    m_bf = _bf16((wq32.T @ wk32) / 32.0)
    ut_bf = _bf16((wo32 @ wv32).T)

    core_ids = list(range(N_CORES))
    in_maps = []
    for b in range(N_CORES):
        in_maps.append({
            "x": np.ascontiguousarray(x[b], dtype=np.float32),
            "m": m_bf,
            "ut": ut_bf,
        })
    res = run_bass_kernel_spmd(nc, in_maps, core_ids)
    return np.stack([res.results[b]["out"] for b in range(N_CORES)], axis=0)
